# revision 1
# baseline (speedup 1.0000x reference)
"""Chamfer distance loss kernel for Trainium2 (Bass/Tile), 8-core data parallel.

Problem: x, y [16, 2048, 3] fp32. Per batch b:
    P[i,j] = |x_i|^2 + |y_j|^2 - 2 x_i.y_j
    loss[b] = mean_j min_i P[i,j] + mean_i min_j P[i,j]

Strategy:
  - Shard batch dim: 2 batches per core across 8 cores, no cross-core comm.
  - P = -2*Q with Q[i,j] = x_i.y_j - 0.5|x_i|^2 - 0.5|y_j|^2, computed as ONE
    K=5 augmented matmul: lhsT rows (x0,x1,x2, -0.5|x|^2, 1), rhs rows
    (y0,y1,y2, 1, -0.5|y|^2).  min P == -2 * max Q.
  - Per [128,512] PSUM tile: row-max (free axis reduce) feeds dr; elementwise
    running max across the 16 m-tiles feeds dl; dl's partition-axis max is done
    by PE-transposing the running-max tiles and free-axis reducing.
  - Final mean via reduce_sum + partition-sum matmul with a ones vector.
"""

import sys

if "/opt/trn_rl_repo" not in sys.path:
    sys.path.insert(0, "/opt/trn_rl_repo")

import numpy as np

B, N, D = 16, 2048, 3
NCORES = 8
BPC = B // NCORES  # batches per core
MT = N // 128  # 16 m-tiles
NT = N // 512  # 4 n-tiles

_CACHE = {}


def _build(matmul_dtype="bfloat16"):
    from contextlib import ExitStack

    import concourse.bass as bass
    import concourse.mybir as mybir
    import concourse.tile as tile
    from concourse import bacc
    from concourse.masks import make_identity

    f32 = mybir.dt.float32
    mm_dt = getattr(mybir.dt, matmul_dtype)

    nc = bacc.Bacc()
    x = nc.dram_tensor("x", [BPC, N, D], f32, kind="ExternalInput")
    y = nc.dram_tensor("y", [BPC, N, D], f32, kind="ExternalInput")
    o = nc.dram_tensor("o", [1, BPC], f32, kind="ExternalOutput")

    with tile.TileContext(nc) as tc, ExitStack() as ctx:
        singles = ctx.enter_context(tc.tile_pool(name="singles", bufs=1))
        aug_pool = ctx.enter_context(tc.tile_pool(name="aug", bufs=2))
        nat_pool = ctx.enter_context(tc.tile_pool(name="nat", bufs=2))
        small_pool = ctx.enter_context(tc.tile_pool(name="small", bufs=3))
        run_pool = ctx.enter_context(tc.tile_pool(name="run", bufs=2))
        col_pool = ctx.enter_context(tc.tile_pool(name="col", bufs=2))
        cp_pool = ctx.enter_context(tc.tile_pool(name="cp", bufs=3))
        mm_psum = ctx.enter_context(tc.tile_pool(name="mmps", bufs=2, space="PSUM"))
        tp_psum = ctx.enter_context(tc.tile_pool(name="tpps", bufs=2, space="PSUM"))
        sc_psum = ctx.enter_context(tc.tile_pool(name="scps", bufs=1, space="PSUM"))

        identity0 = singles.tile([128, 128], f32)
        make_identity(nc, identity0)
        identity = singles.tile([128, 128], mm_dt)
        nc.vector.tensor_copy(identity, identity0)
        ones = singles.tile([128, 1], f32)
        nc.vector.memset(ones, 1.0)
        ones16 = singles.tile([128, N // 128], f32)
        nc.vector.memset(ones16, 1.0)
        out_sb = singles.tile([1, BPC], f32)

        X = mybir.AxisListType.X

        # bf16 triple-split augmented matmul, K=24 rows per operand:
        #   x ~ xh+xm+xl (bf16 levels ~1, 2^-9, 2^-18); kept products
        #   hh,hm,mh,hl,lh,mm give x.y to ~2^-27.  Norms -0.5|x|^2 are
        #   3-way split and paired with ones rows.
        # stage fields (unique, [128,16] each):
        #   3d+0,3d+1,3d+2 = h/m/l of component d; 9,10,11 = norm h/m/l;
        #   12 = ones
        LROWS = []
        RROWS = []
        for d in range(D):
            h, m_, l = 3 * d, 3 * d + 1, 3 * d + 2
            LROWS += [h, h, m_, h, l, m_]
            RROWS += [h, m_, h, l, h, m_]
        LROWS += [9, 10, 11, 12, 12, 12]
        RROWS += [12, 12, 12, 9, 10, 11]
        K = len(LROWS)  # 24

        for b in range(BPC):
            Q = N // 128  # points per partition
            stages = []
            for gi, (side, src) in enumerate((("x", x), ("y", y))):
                stage = nat_pool.tile([128, 13 * Q], mm_dt, tag=f"stage{side}")
                stages.append(stage)
                nat = nat_pool.tile([128, Q * D], f32, tag=f"nat{side}")
                nc.sync.dma_start(
                    out=nat, in_=src[b].rearrange("(p q) d -> p (q d)", p=128)
                )
                sq = nat_pool.tile([128, Q * D], f32, tag=f"sq{side}")
                nc.vector.tensor_mul(sq, nat, nat)
                nrm = small_pool.tile([128, Q], f32, tag=f"nrm{side}")
                nc.vector.tensor_reduce(
                    nrm, sq.rearrange("p (q d) -> p q d", d=D), axis=X,
                    op=mybir.AluOpType.add,
                )
                nc.vector.tensor_scalar_mul(nrm, nrm, -0.5)

                # triple-split of components: stage holds (d,q)-strided views
                stv = stage.rearrange("p (f q) -> p f q", f=13)

                def split3(val_f32, fidx, width, pool_tag):
                    # val_f32: [128, width] fp32; writes bf16 h/m/l into
                    # stage fields fidx, fidx+stride pattern given by caller
                    t1 = nat_pool.tile([128, width], f32, tag=f"{pool_tag}t1")
                    t2 = nat_pool.tile([128, width], f32, tag=f"{pool_tag}t2")
                    nc.vector.tensor_copy(fidx[0], val_f32)          # h
                    nc.vector.tensor_sub(t1, val_f32, fidx[0])
                    nc.vector.tensor_copy(fidx[1], t1)               # m
                    nc.vector.tensor_sub(t2, t1, fidx[1])
                    nc.vector.tensor_copy(fidx[2], t2)               # l

                natv = nat.rearrange("p (q d) -> p d q", d=D)
                for d in range(D):
                    split3(
                        natv[:, d, :],
                        [stv[:, 3 * d + j, :] for j in range(3)],
                        Q, f"c{side}",
                    )
                split3(nrm, [stv[:, 9 + j, :] for j in range(3)], Q, f"n{side}")
                nc.vector.tensor_copy(stv[:, 12, :], ones16)

            # per-row flatten DMAs: aug[r, g*N + p*16+q] = stage_g[p, f*16+q]
            aug = aug_pool.tile([K, 2 * N], mm_dt, tag="aug")
            for g, rows in enumerate((LROWS, RROWS)):
                for r, f in enumerate(rows):
                    nc.sync.dma_start(
                        out=aug[r : r + 1, g * N : (g + 1) * N],
                        in_=stages[g][:, f * Q : (f + 1) * Q],
                    )
            lhsT = aug[:, 0:N]
            rhs = aug[:, N : 2 * N]

            # ---- main loop: Q tiles + reductions ----
            # Per (m, half): 2 matmuls into a 2-bank PSUM group; dr row-max
            # reduced in exact fp32 straight from PSUM; ACT copies the group
            # to bf16 SBUF so the dl running-max chain runs at DVE 2x rate.
            HB = 1024  # half-group width (2 PSUM banks)
            runmax = run_pool.tile([128, N], mm_dt, tag="runmax")
            drcol = col_pool.tile([128, 2 * MT], f32, tag="drcol")
            dlvals = col_pool.tile([128, MT], f32, tag="dlvals")
            for m in range(MT):
                for h in range(2):
                    psg = mm_psum.tile([128, HB], f32, tag="mm")
                    for k in range(2):
                        n = 2 * h + k
                        nc.tensor.matmul(
                            psg[:, k * 512 : (k + 1) * 512],
                            lhsT=lhsT[:, m * 128 : (m + 1) * 128],
                            rhs=rhs[:, n * 512 : (n + 1) * 512],
                            start=True,
                            stop=True,
                        )
                    nc.vector.tensor_reduce(
                        drcol[:, 2 * m + h : 2 * m + h + 1],
                        psg.rearrange("p (a c) -> p a c", a=2),
                        axis=mybir.AxisListType.XY,
                        op=mybir.AluOpType.max,
                    )
                    cp = cp_pool.tile([128, HB], mm_dt, tag="cp")
                    nc.scalar.copy(cp, psg)
                    hsl = slice(h * HB, (h + 1) * HB)
                    if m == 0:
                        nc.vector.tensor_copy(runmax[:, hsl], cp)
                    else:
                        nc.vector.tensor_max(runmax[:, hsl], runmax[:, hsl], cp)

            # ---- dl: partition-axis max via PE transpose (bf16) ----
            for c in range(MT):
                tp = tp_psum.tile([128, 128], mm_dt, tag="tp")
                nc.tensor.transpose(
                    tp, runmax[:, c * 128 : (c + 1) * 128], identity
                )
                nc.vector.reduce_max(dlvals[:, c : c + 1], tp, axis=X)

            # ---- final: mean + partition sum ----
            drm = small_pool.tile([128, MT], f32, tag="drm")
            nc.vector.tensor_reduce(
                drm, drcol.rearrange("p (m a) -> p m a", a=2), axis=X,
                op=mybir.AluOpType.max,
            )
            dlsum = small_pool.tile([128, 1], f32, tag="dlsum")
            drsum = small_pool.tile([128, 1], f32, tag="drsum")
            nc.vector.reduce_sum(dlsum, dlvals, axis=X)
            nc.vector.reduce_sum(drsum, drm, axis=X)
            tot = small_pool.tile([128, 1], f32, tag="tot")
            nc.vector.tensor_add(tot, dlsum, drsum)
            psc = sc_psum.tile([1, 1], f32, tag="psc")
            nc.tensor.matmul(psc, lhsT=tot, rhs=ones, start=True, stop=True)
            nc.vector.tensor_scalar_mul(out_sb[0:1, b : b + 1], psc, -2.0 / N)

        nc.gpsimd.dma_start(out=o[0:1, 0:BPC], in_=out_sb)

    nc.compile()
    return nc


def _get_nc(matmul_dtype="bfloat16"):
    key = matmul_dtype
    if key not in _CACHE:
        _CACHE[key] = _build(matmul_dtype)
    return _CACHE[key]


def kernel(x: np.ndarray, y: np.ndarray) -> np.ndarray:
    from concourse.bass_utils import run_bass_kernel_spmd

    x = np.ascontiguousarray(np.asarray(x, dtype=np.float32))
    y = np.ascontiguousarray(np.asarray(y, dtype=np.float32))
    nc = _get_nc()
    in_maps = [
        {"x": x[c * BPC : (c + 1) * BPC], "y": y[c * BPC : (c + 1) * BPC]}
        for c in range(NCORES)
    ]
    res = run_bass_kernel_spmd(nc, in_maps, core_ids=list(range(NCORES)))
    return np.concatenate([r["o"].reshape(BPC) for r in res.results])

